# revision 1
# baseline (speedup 1.0000x reference)
"""TRN2 Bass kernel for nn_MultiHeadAttention_26156350832790.

Multi-head attention: B=1, S=2048, D=2048, H=16 heads (dk=128), causal mask,
fp32 I/O.  Sharded tensor-parallel over 8 NeuronCores: 2 heads per core.

Per-core dataflow (all matmuls in float32r at full PE rate):
  phase 1: Q^T/K^T [dk, S] and V [S, dk] projections, streaming x^T in
           512-column blocks; Q/K (N=512) interleaved with V (N=256) so
           LDWEIGHTS pipelines under the streams
  phase 2: flash-style attention per (head, 512-wide q-chunk), scores kept
           TRANSPOSED [k, q] so softmax sums come from a ones-matmul and the
           PV matmul needs no P transpose; causal handled by slicing the
           q-range per diagonal k-tile plus one [128,128] triangle mask;
           normalization applied to the accumulated output via a broadcast
           reciprocal multiply
  phase 3: O-projection (contraction over the core's 256 head-dims),
           producing a partial [S, D] summed across cores on the host

Host side: x is pre-transposed, weights pre-tiled into SBUF-friendly
layouts; bq/bk applied in-kernel at Q/K evacuation, bv/bo folded into a
host-side row-vector add (softmax rows sum to 1, so P @ (V + bv) ==
P @ V + bv exactly).
"""

import math
import os
import sys

if "/opt/trn_rl_repo" not in sys.path:
    sys.path.insert(0, "/opt/trn_rl_repo")

import numpy as np

import concourse.bacc as bacc
import concourse.tile as tile
from concourse import mybir
from concourse.bass_utils import run_bass_kernel_spmd

P = 128          # partitions
S = 2048         # sequence
D = 2048         # model dim
NT = 16          # 128-row tiles in S or D
HPC = 2          # heads per core
DK = 128         # head dim
C = 4            # 512-wide chunks
CW = 512         # chunk width
N_CORES = 8
SCALE = 1.0 / math.sqrt(DK)
NEG = -1.0e30

F = mybir.dt.float32
R = mybir.dt.float32r

_NC = None
last_exec_time_ns = None
_last_in_maps = None


def build():
    nc = bacc.Bacc(None)

    xT = nc.dram_tensor("xT", [D, S], R, kind="ExternalInput")
    wq = nc.dram_tensor("wq", [P, NT * 2 * DK], R, kind="ExternalInput")
    wk = nc.dram_tensor("wk", [P, NT * 2 * DK], R, kind="ExternalInput")
    wv = nc.dram_tensor("wv", [P, NT * 2 * DK], R, kind="ExternalInput")
    wo = nc.dram_tensor("wo", [P, HPC * D], R, kind="ExternalInput")
    bqk = nc.dram_tensor("bqk", [P, 4], F, kind="ExternalInput")
    masks = nc.dram_tensor("masks", [P, P], F, kind="ExternalInput")
    onesc = nc.dram_tensor("onesc", [P, 1], R, kind="ExternalInput")
    onesr = nc.dram_tensor("onesr", [1, P], R, kind="ExternalInput")
    out = nc.dram_tensor("out", [S, D], F, kind="ExternalOutput")

    Exp = mybir.ActivationFunctionType.Exp

    with tile.TileContext(nc) as tc:
        with (
            tc.tile_pool(name="consts", bufs=1) as consts,
            tc.tile_pool(name="persist", bufs=1) as persist,
        ):
            bqk_sb = consts.tile([P, 4], F)
            mask_sb = consts.tile([P, P], F)
            ones_col = consts.tile([P, 1], R)
            ones_row = consts.tile([1, P], R)
            nc.sync.dma_start(out=bqk_sb, in_=bqk[:])
            nc.sync.dma_start(out=mask_sb, in_=masks[:])
            nc.sync.dma_start(out=ones_col, in_=onesc[:])
            nc.sync.dma_start(out=ones_row, in_=onesr[:])

            # persistent activations
            qt_sb = persist.tile([P, HPC * S], R)      # Q^T per head [dk, S]
            kt_sb = persist.tile([P, HPC * S], R)      # K^T per head [dk, S]
            v_sb = persist.tile([P, NT * 2 * DK], R)   # V s-tiles [k, 2*dk]
            attnT_sb = persist.tile([P, HPC * S], R)   # attn^T per head [dk, S]

            # ---- phase 1: QKV projections, streaming x^T in 512-col blocks
            xT_tiled = xT.rearrange("(t p) s -> p t s", p=P)
            with (
                tc.tile_pool(name="wqkv", bufs=1) as wqkv,
                tc.tile_pool(name="xin", bufs=2) as xin,
                tc.tile_pool(name="p1ps", bufs=2, space="PSUM") as p1ps,
            ):
                wq_sb = wqkv.tile([P, NT * 2 * DK], R)
                wk_sb = wqkv.tile([P, NT * 2 * DK], R)
                wv_sb = wqkv.tile([P, NT * 2 * DK], R)
                xts = []
                for b in range(C):
                    xtb = xin.tile([P, NT, CW], R, name=f"xt{b}", tag="xt")
                    xts.append(xtb)
                # per-t slices so the first matmuls start ~35us earlier
                for t in range(NT):
                    ws = slice(t * 2 * DK, (t + 1) * 2 * DK)
                    nc.sync.dma_start(out=wq_sb[:, ws], in_=wq[:, ws])
                    nc.sync.dma_start(out=wk_sb[:, ws], in_=wk[:, ws])
                    nc.sync.dma_start(out=wv_sb[:, ws], in_=wv[:, ws])
                    nc.sync.dma_start(
                        out=xts[0][:, t, :], in_=xT_tiled[:, t, 0:CW]
                    )

                for b in range(C):
                    xt = xts[b]
                    if b > 0:
                        for t in range(NT):
                            nc.sync.dma_start(
                                out=xt[:, t, :],
                                in_=xT_tiled[:, t, b * CW : (b + 1) * CW],
                            )
                    for h in range(HPC):
                        qps = p1ps.tile([P, CW], F, name="qps")
                        kps = p1ps.tile([P, CW], F, name="kps")
                        vps0 = p1ps.tile([P, 2 * DK], F, name="vps0")
                        vps1 = p1ps.tile([P, 2 * DK], F, name="vps1")
                        i0 = 2 * h
                        for t in range(NT):
                            st = t == 0
                            sp = t == NT - 1
                            nc.tensor.matmul(
                                qps[:],
                                wq_sb[:, t * 2 * DK + h * DK : t * 2 * DK + (h + 1) * DK],
                                xt[:, t, :],
                                start=st,
                                stop=sp,
                            )
                            nc.tensor.matmul(
                                vps0[:],
                                xt[:, t, i0 * DK : (i0 + 1) * DK],
                                wv_sb[:, t * 2 * DK : (t + 1) * 2 * DK],
                                start=st,
                                stop=sp,
                            )
                            nc.tensor.matmul(
                                kps[:],
                                wk_sb[:, t * 2 * DK + h * DK : t * 2 * DK + (h + 1) * DK],
                                xt[:, t, :],
                                start=st,
                                stop=sp,
                            )
                            nc.tensor.matmul(
                                vps1[:],
                                xt[:, t, (i0 + 1) * DK : (i0 + 2) * DK],
                                wv_sb[:, t * 2 * DK : (t + 1) * 2 * DK],
                                start=st,
                                stop=sp,
                            )
                        nc.scalar.add(
                            qt_sb[:, h * S + b * CW : h * S + (b + 1) * CW],
                            qps[:],
                            bqk_sb[:, h : h + 1],
                        )
                        nc.scalar.add(
                            kt_sb[:, h * S + b * CW : h * S + (b + 1) * CW],
                            kps[:],
                            bqk_sb[:, 2 + h : 3 + h],
                        )
                        u = 4 * b + i0
                        nc.scalar.copy(
                            v_sb[:, u * 2 * DK : (u + 1) * 2 * DK], vps0[:]
                        )
                        nc.scalar.copy(
                            v_sb[:, (u + 1) * 2 * DK : (u + 2) * 2 * DK], vps1[:]
                        )

            # wo loads during phase 2, lives until the end (LIFO pool stack)
            with tc.tile_pool(name="wop", bufs=1) as wop:
                wo_sb = wop.tile([P, HPC * D], R)
                nc.sync.dma_start(out=wo_sb, in_=wo[:])

                # ---- phases 2+3 interleaved per 512-chunk: causal attention
                # (scores transposed [k, q]) then the O-projection for the
                # chunk's s-tiles, so output DMA spreads across the run.
                with (
                    tc.tile_pool(name="ps23", bufs=1, space="PSUM") as ps23,
                    tc.tile_pool(name="ptp", bufs=6) as ptp,
                    tc.tile_pool(name="ssp", bufs=2) as ssp,
                    tc.tile_pool(name="bcp", bufs=2) as bcp,
                    tc.tile_pool(name="outp", bufs=4) as outp,
                ):
                    for c in range(C):
                        for h in range(HPC):
                            jmax = 4 * c + 3
                            sum_ps = ps23.tile([1, CW], F, name="sum_ps", tag="B", bufs=2)
                            o_ps = ps23.tile([P, CW], F, name="o_ps", tag="Cc", bufs=2)
                            lag = None  # (pt, lo, start, stop) pending PV/sums
                            for j in range(jmax + 1):
                                t = j - 4 * c
                                lo = P * t if t >= 0 else 0
                                sc = ps23.tile([P, CW], F, name="sc", tag="A", bufs=3)
                                nc.tensor.matmul(
                                    sc[:, lo:],
                                    kt_sb[:, h * S + j * P : h * S + (j + 1) * P],
                                    qt_sb[:, h * S + c * CW + lo : h * S + (c + 1) * CW],
                                    start=True,
                                    stop=True,
                                )
                                if t >= 0:
                                    nc.vector.tensor_add(
                                        sc[:, lo : lo + P],
                                        sc[:, lo : lo + P],
                                        mask_sb[:],
                                    )
                                pt = ptp.tile([P, CW], R, name="pt")
                                nc.scalar.activation(
                                    pt[:, lo:], sc[:, lo:], Exp, scale=SCALE
                                )
                                if lag is not None:
                                    lpt, llo, lst, lsp, lj = lag
                                    nc.tensor.matmul(
                                        o_ps[:, llo:],
                                        v_sb[:, lj * 2 * DK + h * DK : lj * 2 * DK + (h + 1) * DK],
                                        lpt[:, llo:],
                                        start=lst,
                                        stop=lsp,
                                    )
                                    nc.tensor.matmul(
                                        sum_ps[:, llo:],
                                        ones_col[:],
                                        lpt[:, llo:],
                                        start=lst,
                                        stop=lsp,
                                    )
                                lag = (pt, lo, j == 0, j == jmax, j)
                            lpt, llo, lst, lsp, lj = lag
                            nc.tensor.matmul(
                                o_ps[:, llo:],
                                v_sb[:, lj * 2 * DK + h * DK : lj * 2 * DK + (h + 1) * DK],
                                lpt[:, llo:],
                                start=lst,
                                stop=lsp,
                            )
                            nc.tensor.matmul(
                                sum_ps[:, llo:],
                                ones_col[:],
                                lpt[:, llo:],
                                start=lst,
                                stop=lsp,
                            )
                            ss = ssp.tile([1, CW], R, name="ss")
                            with nc.allow_low_precision(reason="fp32r bcast feed"):
                                nc.scalar.copy(ss[:], sum_ps[:])
                            b_ps = ps23.tile([P, CW], F, name="b_ps", tag="D", bufs=1)
                            nc.tensor.matmul(
                                b_ps[:], ones_row[:], ss[:], start=True, stop=True
                            )
                            bc = bcp.tile([P, CW], F, name="bc")
                            nc.vector.reciprocal(bc[:], b_ps[:])
                            nc.vector.tensor_mul(
                                attnT_sb[:, h * S + c * CW : h * S + (c + 1) * CW],
                                o_ps[:],
                                bc[:],
                            )

                        # O-projection lags one chunk so its matmuls hide the
                        # normalization-chain latency of the current chunk
                        co = c - 1
                        if co < 0:
                            continue
                        for u in range(4 * co, 4 * co + 4):
                            for e in range(C):
                                o3 = ps23.tile([P, CW], F, name="o3", tag="A", bufs=3)
                                for h in range(HPC):
                                    nc.tensor.matmul(
                                        o3[:],
                                        attnT_sb[:, h * S + u * P : h * S + (u + 1) * P],
                                        wo_sb[:, h * D + e * CW : h * D + (e + 1) * CW],
                                        start=(h == 0),
                                        stop=(h == HPC - 1),
                                    )
                                ot = outp.tile([P, CW], F, name="ot")
                                if (u * C + e) % 2 == 0:
                                    nc.scalar.copy(ot[:], o3[:])
                                else:
                                    nc.vector.tensor_copy(ot[:], o3[:])
                                nc.gpsimd.dma_start(
                                    out=out[u * P : (u + 1) * P, e * CW : (e + 1) * CW],
                                    in_=ot[:],
                                )

                    for u in range(4 * (C - 1), 4 * C):
                        for e in range(C):
                            o3 = ps23.tile([P, CW], F, name="o3", tag="A", bufs=3)
                            for h in range(HPC):
                                nc.tensor.matmul(
                                    o3[:],
                                    attnT_sb[:, h * S + u * P : h * S + (u + 1) * P],
                                    wo_sb[:, h * D + e * CW : h * D + (e + 1) * CW],
                                    start=(h == 0),
                                    stop=(h == HPC - 1),
                                )
                            ot = outp.tile([P, CW], F, name="ot")
                            if (u * C + e) % 2 == 0:
                                nc.scalar.copy(ot[:], o3[:])
                            else:
                                nc.vector.tensor_copy(ot[:], o3[:])
                            nc.gpsimd.dma_start(
                                out=out[u * P : (u + 1) * P, e * CW : (e + 1) * CW],
                                in_=ot[:],
                            )

    nc.compile()
    return nc


def _tile_weight_cols(w_slice: np.ndarray) -> np.ndarray:
    """[2048, 256] -> [128, 16*256] with block t = rows [128t, 128t+128)."""
    return np.ascontiguousarray(
        w_slice.reshape(NT, P, 2 * DK).transpose(1, 0, 2).reshape(P, NT * 2 * DK)
    )


def _make_masks() -> np.ndarray:
    """[128,128] additive causal triangle: 0 where p <= f, -1e30 where p > f."""
    p = np.arange(P)[:, None]
    f = np.arange(P)[None, :]
    return np.where(p <= f, 0.0, NEG).astype(np.float32)


def kernel(x, Wq, bq, Wk, bk, Wv, bv, Wo, bo):
    global _NC, last_exec_time_ns, _last_in_maps

    x = np.asarray(x, dtype=np.float32)
    Wq = np.asarray(Wq, dtype=np.float32)
    Wk = np.asarray(Wk, dtype=np.float32)
    Wv = np.asarray(Wv, dtype=np.float32)
    Wo = np.asarray(Wo, dtype=np.float32)
    bq = np.asarray(bq, dtype=np.float32)
    bk = np.asarray(bk, dtype=np.float32)
    bv = np.asarray(bv, dtype=np.float32)
    bo = np.asarray(bo, dtype=np.float32)

    if _NC is None:
        _NC = build()

    xT = np.ascontiguousarray(x[0].T)
    masks = _make_masks()

    in_maps = []
    for i in range(N_CORES):
        cs = slice(2 * DK * i, 2 * DK * (i + 1))
        bqk_i = np.stack(
            [
                bq[2 * DK * i : 2 * DK * i + DK],
                bq[2 * DK * i + DK : 2 * DK * (i + 1)],
                bk[2 * DK * i : 2 * DK * i + DK],
                bk[2 * DK * i + DK : 2 * DK * (i + 1)],
            ],
            axis=1,
        ).astype(np.float32)
        wo_i = np.ascontiguousarray(
            Wo[cs, :].reshape(HPC, P, D).transpose(1, 0, 2).reshape(P, HPC * D)
        )
        in_maps.append(
            {
                "xT": xT,
                "wq": _tile_weight_cols(Wq[:, cs]),
                "wk": _tile_weight_cols(Wk[:, cs]),
                "wv": _tile_weight_cols(Wv[:, cs]),
                "wo": wo_i,
                "bqk": bqk_i,
                "masks": masks,
                "onesc": np.ones((P, 1), np.float32),
                "onesr": np.ones((1, P), np.float32),
            }
        )

    _last_in_maps = in_maps
    trace = bool(int(os.environ.get("BASS_TRACE", "0") or "0"))
    if trace:
        try:
            import ntff_shim

            ntff_shim.install()
        except Exception:
            pass

    res = run_bass_kernel_spmd(
        _NC, in_maps, core_ids=list(range(N_CORES)), trace=trace
    )
    last_exec_time_ns = res.exec_time_ns

    acc = np.zeros((S, D), dtype=np.float64)
    for r_ in res.results:
        acc += r_["out"].astype(np.float64)
    # bv/bo fold: softmax rows sum to 1 => attn @ (V+bv) @ Wo + bo adds bv@Wo + bo
    acc += bv.astype(np.float64) @ Wo.astype(np.float64) + bo.astype(np.float64)
    return acc.astype(np.float32).reshape(1, S, D)



# revision 3
# speedup vs baseline: 1.1321x; 1.1321x over previous
"""TRN2 Bass kernel for nn_MultiHeadAttention_26156350832790.

Multi-head attention: B=1, S=2048, D=2048, H=16 heads (dk=128), causal mask,
fp32 I/O.  Sharded tensor-parallel over 8 NeuronCores: 2 heads per core.

Per-core dataflow (PSUM accumulation always fp32):
  phase 1: full x^T resident in SBUF as bf16 (16 DMAs of [128, 2048] with
           4 KB/partition descriptors); Q^T/K^T [dk, S] evacuated to fp32r,
           V [S, dk] to bf16.  QKV matmuls in bf16.
  phase 2: flash-style attention per (head, 512-wide q-chunk), scores kept
           TRANSPOSED [k, q] (fp32r matmul) so softmax sums come from a
           ones-matmul and the PV matmul needs no P transpose; exp on the
           scalar engine to bf16 probs; causal handled by q-range slicing
           plus one [128,128] triangle mask; normalization via ones-row
           broadcast matmul + reciprocal_approx_fast + vector multiply.
  phase 3: O-projection in bf16 (contraction over the core's 256 head-dims)
           interleaved per chunk, lagging attention by one chunk; partial
           [S, D] written out in bf16 and summed across cores on the host.

Host side: x pre-transposed+tiled, weights pre-tiled, everything cast to
bf16; bq/bk applied in-kernel at Q/K evacuation, bv/bo folded into a
host-side row-vector add (softmax rows sum to 1, so P @ (V + bv) ==
P @ V + bv exactly).
"""

import math
import os
import sys

if "/opt/trn_rl_repo" not in sys.path:
    sys.path.insert(0, "/opt/trn_rl_repo")

import numpy as np
import ml_dtypes

import concourse.bacc as bacc
import concourse.tile as tile
from concourse import mybir
from concourse.bass_utils import run_bass_kernel_spmd

P = 128          # partitions
S = 2048         # sequence
D = 2048         # model dim
NT = 16          # 128-row tiles in S or D
HPC = 2          # heads per core
DK = 128         # head dim
C = 4            # 512-wide chunks
CW = 512         # chunk width
N_CORES = 8
SCALE = 1.0 / math.sqrt(DK)
NEG = -1.0e30

F = mybir.dt.float32
R = mybir.dt.float32r
BF = mybir.dt.bfloat16

_NC = None
last_exec_time_ns = None
_last_in_maps = None


def build():
    nc = bacc.Bacc(None)

    xt = nc.dram_tensor("xt", [P, NT * S], BF, kind="ExternalInput")
    wq = nc.dram_tensor("wq", [P, NT * 2 * DK], BF, kind="ExternalInput")
    wk = nc.dram_tensor("wk", [P, NT * 2 * DK], BF, kind="ExternalInput")
    wv = nc.dram_tensor("wv", [P, NT * 2 * DK], BF, kind="ExternalInput")
    wo = nc.dram_tensor("wo", [P, HPC * D], BF, kind="ExternalInput")
    bqk = nc.dram_tensor("bqk", [P, 4], F, kind="ExternalInput")
    masks = nc.dram_tensor("masks", [P, P], F, kind="ExternalInput")
    onesc = nc.dram_tensor("onesc", [P, 1], BF, kind="ExternalInput")
    onesr = nc.dram_tensor("onesr", [1, P], R, kind="ExternalInput")
    out = nc.dram_tensor("out", [P, NT * D], BF, kind="ExternalOutput")

    Exp = mybir.ActivationFunctionType.Exp

    with tile.TileContext(nc) as tc:
        with (
            tc.tile_pool(name="consts", bufs=1) as consts,
            tc.tile_pool(name="persist", bufs=1) as persist,
        ):
            bqk_sb = consts.tile([P, 4], F)
            mask_sb = consts.tile([P, P], F)
            ones_col = consts.tile([P, 1], BF)
            ones_row = consts.tile([1, P], R)
            nc.sync.dma_start(out=bqk_sb, in_=bqk[:])
            nc.sync.dma_start(out=mask_sb, in_=masks[:])
            nc.sync.dma_start(out=ones_col, in_=onesc[:])
            nc.sync.dma_start(out=ones_row, in_=onesr[:])

            # persistent activations
            x_sb = persist.tile([P, NT, S], BF)        # full x^T, t-tiled
            qt_sb = persist.tile([P, HPC * S], R)      # Q^T per head [dk, S]
            kt_sb = persist.tile([P, HPC * S], R)      # K^T per head [dk, S]
            v_sb = persist.tile([P, NT * 2 * DK], BF)  # V s-tiles [k, 2*dk]
            attnT_sb = persist.tile([P, HPC * S], BF)  # attn^T per head [dk, S]

            # ---- phase 1: QKV projections off SBUF-resident x^T
            with (
                tc.tile_pool(name="wqkv", bufs=1) as wqkv,
                tc.tile_pool(name="p1ps", bufs=2, space="PSUM") as p1ps,
            ):
                wq_sb = wqkv.tile([P, NT * 2 * DK], BF)
                wk_sb = wqkv.tile([P, NT * 2 * DK], BF)
                wv_sb = wqkv.tile([P, NT * 2 * DK], BF)
                # DMA order: x[0] first, then weights (halves so the first
                # matmuls can start early), then the rest of x
                HW = NT * DK  # half the weight columns
                nc.sync.dma_start(out=x_sb[:, 0, :], in_=xt[:, 0:S])
                nc.sync.dma_start(out=wq_sb[:, :HW], in_=wq[:, :HW])
                nc.sync.dma_start(out=wk_sb[:, :HW], in_=wk[:, :HW])
                nc.sync.dma_start(out=wv_sb[:, :HW], in_=wv[:, :HW])
                nc.sync.dma_start(out=wq_sb[:, HW:], in_=wq[:, HW:])
                nc.sync.dma_start(out=wk_sb[:, HW:], in_=wk[:, HW:])
                nc.sync.dma_start(out=wv_sb[:, HW:], in_=wv[:, HW:])
                for t in range(1, NT):
                    nc.sync.dma_start(
                        out=x_sb[:, t, :], in_=xt[:, t * S : (t + 1) * S]
                    )

                for b in range(C):
                    for h in range(HPC):
                        qps = p1ps.tile([P, CW], F, name="qps")
                        kps = p1ps.tile([P, CW], F, name="kps")
                        vps0 = p1ps.tile([P, 2 * DK], F, name="vps0")
                        vps1 = p1ps.tile([P, 2 * DK], F, name="vps1")
                        i0 = 2 * h
                        u = 4 * b + i0
                        for t in range(NT):
                            st = t == 0
                            sp = t == NT - 1
                            cs = slice(b * CW, (b + 1) * CW)
                            nc.tensor.matmul(
                                qps[:],
                                wq_sb[:, t * 2 * DK + h * DK : t * 2 * DK + (h + 1) * DK],
                                x_sb[:, t, cs],
                                start=st,
                                stop=sp,
                            )
                            nc.tensor.matmul(
                                vps0[:],
                                x_sb[:, t, u * DK : (u + 1) * DK],
                                wv_sb[:, t * 2 * DK : (t + 1) * 2 * DK],
                                start=st,
                                stop=sp,
                            )
                            nc.tensor.matmul(
                                kps[:],
                                wk_sb[:, t * 2 * DK + h * DK : t * 2 * DK + (h + 1) * DK],
                                x_sb[:, t, cs],
                                start=st,
                                stop=sp,
                            )
                            nc.tensor.matmul(
                                vps1[:],
                                x_sb[:, t, (u + 1) * DK : (u + 2) * DK],
                                wv_sb[:, t * 2 * DK : (t + 1) * 2 * DK],
                                start=st,
                                stop=sp,
                            )
                        with nc.allow_low_precision(reason="fp32r evac"):
                            nc.scalar.add(
                                qt_sb[:, h * S + b * CW : h * S + (b + 1) * CW],
                                qps[:],
                                bqk_sb[:, h : h + 1],
                            )
                            nc.scalar.add(
                                kt_sb[:, h * S + b * CW : h * S + (b + 1) * CW],
                                kps[:],
                                bqk_sb[:, 2 + h : 3 + h],
                            )
                        with nc.allow_low_precision(reason="bf16 V evac"):
                            nc.vector.tensor_copy(
                                v_sb[:, u * 2 * DK : (u + 1) * 2 * DK], vps0[:]
                            )
                            nc.vector.tensor_copy(
                                v_sb[:, (u + 1) * 2 * DK : (u + 2) * 2 * DK],
                                vps1[:],
                            )

            # wo loads during phase 2, lives until the end (LIFO pool stack)
            with tc.tile_pool(name="wop", bufs=1) as wop:
                wo_sb = wop.tile([P, HPC * D], BF)
                nc.sync.dma_start(out=wo_sb, in_=wo[:])

                # ---- phases 2+3 interleaved per 512-chunk: causal attention
                # (scores transposed [k, q]) then the O-projection for the
                # previous chunk's s-tiles.
                with (
                    tc.tile_pool(name="ps23", bufs=1, space="PSUM") as ps23,
                    tc.tile_pool(name="ptp", bufs=4) as ptp,
                    tc.tile_pool(name="ssp", bufs=2) as ssp,
                    tc.tile_pool(name="bcp", bufs=2) as bcp,
                    tc.tile_pool(name="outp", bufs=2) as outp,
                ):

                    def oproj(co):
                        for up in range(2):
                            u0 = 4 * co + 2 * up
                            ot = outp.tile([P, 2 * D], BF, name="ot", tag="ot")
                            for du in range(2):
                                u = u0 + du
                                for e in range(C):
                                    o3 = ps23.tile(
                                        [P, CW], F, name="o3", tag="o3", bufs=2
                                    )
                                    for h in range(HPC):
                                        nc.tensor.matmul(
                                            o3[:],
                                            attnT_sb[:, h * S + u * P : h * S + (u + 1) * P],
                                            wo_sb[:, h * D + e * CW : h * D + (e + 1) * CW],
                                            start=(h == 0),
                                            stop=(h == HPC - 1),
                                        )
                                    with nc.allow_low_precision(reason="bf16 out"):
                                        nc.vector.tensor_copy(
                                            ot[:, du * D + e * CW : du * D + (e + 1) * CW],
                                            o3[:],
                                        )
                            nc.scalar.dma_start(
                                out=out[:, u0 * D : (u0 + 2) * D],
                                in_=ot[:],
                            )

                    for c in range(C):
                        for h in range(HPC):
                            jmax = 4 * c + 3
                            sum_ps = ps23.tile(
                                [1, CW], F, name="sum_ps", tag="B", bufs=1
                            )
                            o_ps = ps23.tile(
                                [P, CW], F, name="o_ps", tag="Cc", bufs=2
                            )
                            lag = None  # pending PV/sums
                            for j in range(jmax + 1):
                                t = j - 4 * c
                                lo = P * t if t >= 0 else 0
                                sc = ps23.tile(
                                    [P, CW], F, name="sc", tag="A", bufs=2
                                )
                                nc.tensor.matmul(
                                    sc[:, lo:],
                                    kt_sb[:, h * S + j * P : h * S + (j + 1) * P],
                                    qt_sb[:, h * S + c * CW + lo : h * S + (c + 1) * CW],
                                    start=True,
                                    stop=True,
                                )
                                if t >= 0:
                                    nc.vector.tensor_add(
                                        sc[:, lo : lo + P],
                                        sc[:, lo : lo + P],
                                        mask_sb[:],
                                    )
                                pt = ptp.tile([P, CW], BF, name="pt")
                                nc.scalar.activation(
                                    pt[:, lo:], sc[:, lo:], Exp, scale=SCALE
                                )
                                if lag is not None:
                                    lpt, llo, lst, lsp, lj = lag
                                    nc.tensor.matmul(
                                        o_ps[:, llo:],
                                        v_sb[:, lj * 2 * DK + h * DK : lj * 2 * DK + (h + 1) * DK],
                                        lpt[:, llo:],
                                        start=lst,
                                        stop=lsp,
                                    )
                                    nc.tensor.matmul(
                                        sum_ps[:, llo:],
                                        ones_col[:],
                                        lpt[:, llo:],
                                        start=lst,
                                        stop=lsp,
                                    )
                                lag = (pt, lo, j == 0, j == jmax, j)
                            lpt, llo, lst, lsp, lj = lag
                            nc.tensor.matmul(
                                o_ps[:, llo:],
                                v_sb[:, lj * 2 * DK + h * DK : lj * 2 * DK + (h + 1) * DK],
                                lpt[:, llo:],
                                start=lst,
                                stop=lsp,
                            )
                            nc.tensor.matmul(
                                sum_ps[:, llo:],
                                ones_col[:],
                                lpt[:, llo:],
                                start=lst,
                                stop=lsp,
                            )
                            # normalization: 1/rowsum broadcast to [P, CW]
                            ss = ssp.tile([1, CW], R, name="ss")
                            with nc.allow_low_precision(reason="fp32r bcast feed"):
                                nc.vector.tensor_copy(ss[:], sum_ps[:])
                            b_ps = ps23.tile(
                                [P, CW], F, name="b_ps", tag="Dd", bufs=1
                            )
                            nc.tensor.matmul(
                                b_ps[:], ones_row[:], ss[:], start=True, stop=True
                            )
                            bc = bcp.tile([P, CW], F, name="bc")
                            nc.vector.reciprocal_approx_fast(out=bc[:], in_=b_ps[:])
                            with nc.allow_low_precision(reason="bf16 attnT"):
                                nc.vector.tensor_mul(
                                    attnT_sb[:, h * S + c * CW : h * S + (c + 1) * CW],
                                    o_ps[:],
                                    bc[:],
                                )

                        # O-projection lags attention by one chunk
                        if c > 0:
                            oproj(c - 1)
                    oproj(C - 1)

    nc.compile()
    return nc


def _tile_weight_cols(w_slice: np.ndarray) -> np.ndarray:
    """[2048, 256] -> [128, 16*256] with block t = rows [128t, 128t+128)."""
    return np.ascontiguousarray(
        w_slice.reshape(NT, P, 2 * DK).transpose(1, 0, 2).reshape(P, NT * 2 * DK)
    )


def _make_masks() -> np.ndarray:
    """[128,128] additive causal triangle: 0 where p <= f, -1e30 where p > f."""
    p = np.arange(P)[:, None]
    f = np.arange(P)[None, :]
    return np.where(p <= f, 0.0, NEG).astype(np.float32)


def kernel(x, Wq, bq, Wk, bk, Wv, bv, Wo, bo):
    global _NC, last_exec_time_ns, _last_in_maps

    BFH = ml_dtypes.bfloat16
    x = np.asarray(x, dtype=np.float32)
    Wq = np.asarray(Wq, dtype=np.float32)
    Wk = np.asarray(Wk, dtype=np.float32)
    Wv = np.asarray(Wv, dtype=np.float32)
    Wo = np.asarray(Wo, dtype=np.float32)
    bq = np.asarray(bq, dtype=np.float32)
    bk = np.asarray(bk, dtype=np.float32)
    bv = np.asarray(bv, dtype=np.float32)
    bo = np.asarray(bo, dtype=np.float32)

    if _NC is None:
        _NC = build()

    # x^T tiled: xt[p, t*S + s] = x[s, t*128 + p]
    xt = np.ascontiguousarray(
        x[0].T.reshape(NT, P, S).transpose(1, 0, 2).reshape(P, NT * S)
    ).astype(BFH)
    masks = _make_masks()

    in_maps = []
    for i in range(N_CORES):
        cs = slice(2 * DK * i, 2 * DK * (i + 1))
        bqk_i = np.stack(
            [
                bq[2 * DK * i : 2 * DK * i + DK],
                bq[2 * DK * i + DK : 2 * DK * (i + 1)],
                bk[2 * DK * i : 2 * DK * i + DK],
                bk[2 * DK * i + DK : 2 * DK * (i + 1)],
            ],
            axis=1,
        ).astype(np.float32)
        wo_i = np.ascontiguousarray(
            Wo[cs, :].reshape(HPC, P, D).transpose(1, 0, 2).reshape(P, HPC * D)
        ).astype(BFH)
        in_maps.append(
            {
                "xt": xt,
                "wq": _tile_weight_cols(Wq[:, cs]).astype(BFH),
                "wk": _tile_weight_cols(Wk[:, cs]).astype(BFH),
                "wv": _tile_weight_cols(Wv[:, cs]).astype(BFH),
                "wo": wo_i,
                "bqk": bqk_i,
                "masks": masks,
                "onesc": np.ones((P, 1), BFH),
                "onesr": np.ones((1, P), np.float32),
            }
        )

    _last_in_maps = in_maps
    trace = bool(int(os.environ.get("BASS_TRACE", "0") or "0"))
    if trace:
        try:
            import ntff_shim

            ntff_shim.install()
        except Exception:
            pass

    res = run_bass_kernel_spmd(
        _NC, in_maps, core_ids=list(range(N_CORES)), trace=trace
    )
    last_exec_time_ns = res.exec_time_ns

    acc = np.zeros((S, D), dtype=np.float64)
    for r_ in res.results:
        part = np.asarray(r_["out"]).astype(np.float64)
        # out[p, u*D + col] = partial[u*128 + p, col]
        acc += part.reshape(P, NT, D).transpose(1, 0, 2).reshape(S, D)
    # bv/bo fold: softmax rows sum to 1 => attn @ (V+bv) @ Wo + bo adds bv@Wo + bo
    acc += bv.astype(np.float64) @ Wo.astype(np.float64) + bo.astype(np.float64)
    return acc.astype(np.float32).reshape(1, S, D)


# revision 6
# speedup vs baseline: 1.1535x; 1.0189x over previous
"""TRN2 Bass kernel for nn_MultiHeadAttention_26156350832790.

Multi-head attention: B=1, S=2048, D=2048, H=16 heads (dk=128), causal mask,
fp32 I/O.  Sharded tensor-parallel over 8 NeuronCores: 2 heads per core.

Per-core dataflow (PSUM accumulation always fp32):
  phase 1: full x^T resident in SBUF as bf16 (16 DMAs of [128, 2048] with
           4 KB/partition descriptors); Q^T/K^T [dk, S] evacuated to fp32r,
           V [S, dk] to bf16.  QKV matmuls in bf16.
  phase 2: flash-style attention per (head, 512-wide q-chunk), scores kept
           TRANSPOSED [k, q] (fp32r matmul) so softmax sums come from a
           ones-matmul and the PV matmul needs no P transpose; exp on the
           scalar engine to bf16 probs; causal handled by q-range slicing
           plus one [128,128] triangle mask; normalization via ones-row
           broadcast matmul + reciprocal_approx_fast + vector multiply.
  phase 3: O-projection in bf16 (contraction over the core's 256 head-dims)
           interleaved per chunk, lagging attention by one chunk; partial
           [S, D] written out in bf16 and summed across cores on the host.

Host side: x pre-transposed+tiled, weights pre-tiled, everything cast to
bf16; bq/bk applied in-kernel at Q/K evacuation, bv/bo folded into a
host-side row-vector add (softmax rows sum to 1, so P @ (V + bv) ==
P @ V + bv exactly).
"""

import math
import os
import sys

if "/opt/trn_rl_repo" not in sys.path:
    sys.path.insert(0, "/opt/trn_rl_repo")

import numpy as np
import ml_dtypes

import concourse.bacc as bacc
import concourse.tile as tile
from concourse import mybir
from concourse.bass_utils import run_bass_kernel_spmd

P = 128          # partitions
S = 2048         # sequence
D = 2048         # model dim
NT = 16          # 128-row tiles in S or D
HPC = 2          # heads per core
DK = 128         # head dim
C = 4            # 512-wide chunks
CW = 512         # chunk width
N_CORES = 8
SCALE = 1.0 / math.sqrt(DK)
NEG = -1.0e30

F = mybir.dt.float32
R = mybir.dt.float32r
BF = mybir.dt.bfloat16

_NC = None
last_exec_time_ns = None
_last_in_maps = None


def build():
    nc = bacc.Bacc(None)

    xt = nc.dram_tensor("xt", [P, NT * S], BF, kind="ExternalInput")
    wq = nc.dram_tensor("wq", [P, NT * 2 * DK], BF, kind="ExternalInput")
    wk = nc.dram_tensor("wk", [P, NT * 2 * DK], BF, kind="ExternalInput")
    wv = nc.dram_tensor("wv", [P, NT * 2 * DK], BF, kind="ExternalInput")
    wo = nc.dram_tensor("wo", [P, HPC * D], BF, kind="ExternalInput")
    bqk = nc.dram_tensor("bqk", [P, 4], F, kind="ExternalInput")
    masks = nc.dram_tensor("masks", [P, P], F, kind="ExternalInput")
    onesc = nc.dram_tensor("onesc", [P, 1], BF, kind="ExternalInput")
    onesr = nc.dram_tensor("onesr", [1, P], R, kind="ExternalInput")
    out = nc.dram_tensor("out", [P, NT * D], BF, kind="ExternalOutput")

    Exp = mybir.ActivationFunctionType.Exp

    with tile.TileContext(nc) as tc:
        with (
            tc.tile_pool(name="consts", bufs=1) as consts,
            tc.tile_pool(name="persist", bufs=1) as persist,
        ):
            bqk_sb = consts.tile([P, 4], F)
            mask_sb = consts.tile([P, P], F)
            ones_col = consts.tile([P, 1], BF)
            ones_row = consts.tile([1, P], R)
            # const DMAs are issued inside phase 1's ordered DMA sequence

            # persistent activations
            x_sb = persist.tile([P, NT, S], BF)        # full x^T, t-tiled
            qt_sb = persist.tile([P, HPC * S], R)      # Q^T per head [dk, S]
            kt_sb = persist.tile([P, HPC * S], R)      # K^T per head [dk, S]
            v_sb = persist.tile([P, NT * 2 * DK], BF)  # V s-tiles [k, 2*dk]
            attnT_sb = persist.tile([P, HPC * S], BF)  # attn^T per head [dk, S]
            wo_sb = persist.tile([P, HPC * D], BF)     # loaded early, used late

            # ---- phase 1: QKV projections off SBUF-resident x^T
            with (
                tc.tile_pool(name="wqkv", bufs=1) as wqkv,
                tc.tile_pool(name="p1ps", bufs=1, space="PSUM") as p1ps,
            ):
                wq_sb = wqkv.tile([P, NT * 2 * DK], BF)
                wk_sb = wqkv.tile([P, NT * 2 * DK], BF)
                wv_sb = wqkv.tile([P, NT * 2 * DK], BF)
                # DMA order tuned so the first matmuls can start early and
                # the x stream stays ahead of the h-fused t-loop.
                HW = NT * DK  # half the weight columns
                nc.sync.dma_start(out=x_sb[:, 0, :], in_=xt[:, 0:S])
                nc.sync.dma_start(out=wq_sb[:, :HW], in_=wq[:, :HW])
                nc.sync.dma_start(out=wk_sb[:, :HW], in_=wk[:, :HW])
                nc.sync.dma_start(out=wv_sb[:, :HW], in_=wv[:, :HW])
                nc.sync.dma_start(out=bqk_sb, in_=bqk[:])
                nc.sync.dma_start(out=x_sb[:, 1, :], in_=xt[:, S : 2 * S])
                nc.sync.dma_start(out=wq_sb[:, HW:], in_=wq[:, HW:])
                nc.sync.dma_start(out=wk_sb[:, HW:], in_=wk[:, HW:])
                nc.sync.dma_start(out=wv_sb[:, HW:], in_=wv[:, HW:])
                nc.sync.dma_start(out=x_sb[:, 2, :], in_=xt[:, 2 * S : 3 * S])
                nc.sync.dma_start(out=wo_sb, in_=wo[:])
                nc.sync.dma_start(out=mask_sb, in_=masks[:])
                nc.sync.dma_start(out=ones_col, in_=onesc[:])
                nc.sync.dma_start(out=ones_row, in_=onesr[:])
                for t in range(3, NT):
                    nc.sync.dma_start(
                        out=x_sb[:, t, :], in_=xt[:, t * S : (t + 1) * S]
                    )

                for b in range(C):
                    # h-fused t-loop: both heads consume x[t] as it lands,
                    # 8 matmuls per tile so compute outpaces the DMA stream
                    ps = {}
                    for h in range(HPC):
                        ps[h] = (
                            p1ps.tile([P, CW], F, name=f"qps{h}", tag=f"qps{h}"),
                            p1ps.tile([P, CW], F, name=f"kps{h}", tag=f"kps{h}"),
                            p1ps.tile([P, 2 * DK], F, name=f"vps{h}0", tag=f"vps{h}0"),
                            p1ps.tile([P, 2 * DK], F, name=f"vps{h}1", tag=f"vps{h}1"),
                        )
                    for t in range(NT):
                        st = t == 0
                        sp = t == NT - 1
                        cs = slice(b * CW, (b + 1) * CW)
                        for h in range(HPC):
                            qps, kps, vps0, vps1 = ps[h]
                            u = 4 * b + 2 * h
                            nc.tensor.matmul(
                                qps[:],
                                wq_sb[:, t * 2 * DK + h * DK : t * 2 * DK + (h + 1) * DK],
                                x_sb[:, t, cs],
                                start=st,
                                stop=sp,
                            )
                            nc.tensor.matmul(
                                vps0[:],
                                x_sb[:, t, u * DK : (u + 1) * DK],
                                wv_sb[:, t * 2 * DK : (t + 1) * 2 * DK],
                                start=st,
                                stop=sp,
                            )
                            nc.tensor.matmul(
                                kps[:],
                                wk_sb[:, t * 2 * DK + h * DK : t * 2 * DK + (h + 1) * DK],
                                x_sb[:, t, cs],
                                start=st,
                                stop=sp,
                            )
                            nc.tensor.matmul(
                                vps1[:],
                                x_sb[:, t, (u + 1) * DK : (u + 2) * DK],
                                wv_sb[:, t * 2 * DK : (t + 1) * 2 * DK],
                                start=st,
                                stop=sp,
                            )
                    for h in range(HPC):
                        qps, kps, vps0, vps1 = ps[h]
                        u = 4 * b + 2 * h
                        with nc.allow_low_precision(reason="fp32r evac"):
                            nc.scalar.add(
                                qt_sb[:, h * S + b * CW : h * S + (b + 1) * CW],
                                qps[:],
                                bqk_sb[:, h : h + 1],
                            )
                            nc.scalar.add(
                                kt_sb[:, h * S + b * CW : h * S + (b + 1) * CW],
                                kps[:],
                                bqk_sb[:, 2 + h : 3 + h],
                            )
                        with nc.allow_low_precision(reason="bf16 V evac"):
                            nc.vector.tensor_copy(
                                v_sb[:, u * 2 * DK : (u + 1) * 2 * DK], vps0[:]
                            )
                            nc.vector.tensor_copy(
                                v_sb[:, (u + 1) * 2 * DK : (u + 2) * 2 * DK],
                                vps1[:],
                            )

            # ---- phases 2+3: causal attention (scores transposed [k, q])
            # with the previous chunk's O-projection units interleaved at
            # j-tile granularity so the in-order tensor queue always has
            # ready work while the scalar engine paces the exp chain.
            with (
                tc.tile_pool(name="ps23", bufs=1, space="PSUM") as ps23,
                tc.tile_pool(name="ptp", bufs=4) as ptp,
                tc.tile_pool(name="ssp", bufs=2) as ssp,
                tc.tile_pool(name="bcp", bufs=2) as bcp,
                tc.tile_pool(name="outp", bufs=3) as outp,
            ):
                ot_cur = [None]

                def emit_ounit(u, e):
                    if e == 0:
                        ot_cur[0] = outp.tile([P, D], BF, name="ot", tag="ot")
                    ot = ot_cur[0]
                    o3 = ps23.tile([P, CW], F, name="o3", tag="o3", bufs=2)
                    for h in range(HPC):
                        nc.tensor.matmul(
                            o3[:],
                            attnT_sb[:, h * S + u * P : h * S + (u + 1) * P],
                            wo_sb[:, h * D + e * CW : h * D + (e + 1) * CW],
                            start=(h == 0),
                            stop=(h == HPC - 1),
                        )
                    with nc.allow_low_precision(reason="bf16 out"):
                        nc.vector.tensor_copy(
                            ot[:, e * CW : (e + 1) * CW], o3[:]
                        )
                    if e == C - 1:
                        nc.scalar.dma_start(
                            out=out[:, u * D : (u + 1) * D], in_=ot[:]
                        )

                for c in range(C):
                    # O-units of the previous chunk, spread over this chunk
                    units = (
                        [(u, e) for u in range(4 * (c - 1), 4 * c) for e in range(C)]
                        if c > 0
                        else []
                    )
                    nslots = HPC * (4 * c + 4)
                    slot = 0
                    emitted = 0

                    def pace():
                        nonlocal slot, emitted
                        slot += 1
                        want = (len(units) * slot) // nslots
                        while emitted < want:
                            emit_ounit(*units[emitted])
                            emitted += 1

                    for h in range(HPC):
                        jmax = 4 * c + 3
                        sum_ps = ps23.tile(
                            [1, CW], F, name="sum_ps", tag="B", bufs=1
                        )
                        o_ps = ps23.tile(
                            [P, CW], F, name="o_ps", tag="Cc", bufs=2
                        )
                        lag = None  # pending PV/sums
                        for j in range(jmax + 1):
                            t = j - 4 * c
                            lo = P * t if t >= 0 else 0
                            sc = ps23.tile(
                                [P, CW], F, name="sc", tag="A", bufs=2
                            )
                            nc.tensor.matmul(
                                sc[:, lo:],
                                kt_sb[:, h * S + j * P : h * S + (j + 1) * P],
                                qt_sb[:, h * S + c * CW + lo : h * S + (c + 1) * CW],
                                start=True,
                                stop=True,
                            )
                            if t >= 0:
                                nc.vector.tensor_add(
                                    sc[:, lo : lo + P],
                                    sc[:, lo : lo + P],
                                    mask_sb[:],
                                )
                            pt = ptp.tile([P, CW], BF, name="pt")
                            nc.scalar.activation(
                                pt[:, lo:], sc[:, lo:], Exp, scale=SCALE
                            )
                            if lag is not None:
                                lpt, llo, lst, lsp, lj = lag
                                nc.tensor.matmul(
                                    o_ps[:, llo:],
                                    v_sb[:, lj * 2 * DK + h * DK : lj * 2 * DK + (h + 1) * DK],
                                    lpt[:, llo:],
                                    start=lst,
                                    stop=lsp,
                                )
                                nc.tensor.matmul(
                                    sum_ps[:, llo:],
                                    ones_col[:],
                                    lpt[:, llo:],
                                    start=lst,
                                    stop=lsp,
                                )
                            lag = (pt, lo, j == 0, j == jmax, j)
                            pace()
                        lpt, llo, lst, lsp, lj = lag
                        nc.tensor.matmul(
                            o_ps[:, llo:],
                            v_sb[:, lj * 2 * DK + h * DK : lj * 2 * DK + (h + 1) * DK],
                            lpt[:, llo:],
                            start=lst,
                            stop=lsp,
                        )
                        nc.tensor.matmul(
                            sum_ps[:, llo:],
                            ones_col[:],
                            lpt[:, llo:],
                            start=lst,
                            stop=lsp,
                        )
                        # normalization: 1/rowsum broadcast to [P, CW]
                        ss = ssp.tile([1, CW], R, name="ss")
                        with nc.allow_low_precision(reason="fp32r bcast feed"):
                            nc.vector.tensor_copy(ss[:], sum_ps[:])
                        b_ps = ps23.tile(
                            [P, CW], F, name="b_ps", tag="Dd", bufs=1
                        )
                        nc.tensor.matmul(
                            b_ps[:], ones_row[:], ss[:], start=True, stop=True
                        )
                        bc = bcp.tile([P, CW], F, name="bc")
                        nc.vector.reciprocal_approx_fast(out=bc[:], in_=b_ps[:])
                        with nc.allow_low_precision(reason="bf16 attnT"):
                            nc.vector.tensor_mul(
                                attnT_sb[:, h * S + c * CW : h * S + (c + 1) * CW],
                                o_ps[:],
                                bc[:],
                            )
                    while emitted < len(units):
                        emit_ounit(*units[emitted])
                        emitted += 1
                # final chunk's O-projection
                for u in range(4 * (C - 1), 4 * C):
                    for e in range(C):
                        emit_ounit(u, e)

    nc.compile()
    return nc


def _tile_weight_cols(w_slice: np.ndarray) -> np.ndarray:
    """[2048, 256] -> [128, 16*256] with block t = rows [128t, 128t+128)."""
    return np.ascontiguousarray(
        w_slice.reshape(NT, P, 2 * DK).transpose(1, 0, 2).reshape(P, NT * 2 * DK)
    )


def _make_masks() -> np.ndarray:
    """[128,128] additive causal triangle: 0 where p <= f, -1e30 where p > f."""
    p = np.arange(P)[:, None]
    f = np.arange(P)[None, :]
    return np.where(p <= f, 0.0, NEG).astype(np.float32)


def kernel(x, Wq, bq, Wk, bk, Wv, bv, Wo, bo):
    global _NC, last_exec_time_ns, _last_in_maps

    BFH = ml_dtypes.bfloat16
    x = np.asarray(x, dtype=np.float32)
    Wq = np.asarray(Wq, dtype=np.float32)
    Wk = np.asarray(Wk, dtype=np.float32)
    Wv = np.asarray(Wv, dtype=np.float32)
    Wo = np.asarray(Wo, dtype=np.float32)
    bq = np.asarray(bq, dtype=np.float32)
    bk = np.asarray(bk, dtype=np.float32)
    bv = np.asarray(bv, dtype=np.float32)
    bo = np.asarray(bo, dtype=np.float32)

    if _NC is None:
        _NC = build()

    # x^T tiled: xt[p, t*S + s] = x[s, t*128 + p]
    xt = np.ascontiguousarray(
        x[0].T.reshape(NT, P, S).transpose(1, 0, 2).reshape(P, NT * S)
    ).astype(BFH)
    masks = _make_masks()

    in_maps = []
    for i in range(N_CORES):
        cs = slice(2 * DK * i, 2 * DK * (i + 1))
        bqk_i = np.stack(
            [
                bq[2 * DK * i : 2 * DK * i + DK],
                bq[2 * DK * i + DK : 2 * DK * (i + 1)],
                bk[2 * DK * i : 2 * DK * i + DK],
                bk[2 * DK * i + DK : 2 * DK * (i + 1)],
            ],
            axis=1,
        ).astype(np.float32)
        wo_i = np.ascontiguousarray(
            Wo[cs, :].reshape(HPC, P, D).transpose(1, 0, 2).reshape(P, HPC * D)
        ).astype(BFH)
        in_maps.append(
            {
                "xt": xt,
                "wq": _tile_weight_cols(Wq[:, cs]).astype(BFH),
                "wk": _tile_weight_cols(Wk[:, cs]).astype(BFH),
                "wv": _tile_weight_cols(Wv[:, cs]).astype(BFH),
                "wo": wo_i,
                "bqk": bqk_i,
                "masks": masks,
                "onesc": np.ones((P, 1), BFH),
                "onesr": np.ones((1, P), np.float32),
            }
        )

    _last_in_maps = in_maps
    trace = bool(int(os.environ.get("BASS_TRACE", "0") or "0"))
    if trace:
        try:
            import ntff_shim

            ntff_shim.install()
        except Exception:
            pass

    res = run_bass_kernel_spmd(
        _NC, in_maps, core_ids=list(range(N_CORES)), trace=trace
    )
    last_exec_time_ns = res.exec_time_ns

    acc = np.zeros((S, D), dtype=np.float64)
    for r_ in res.results:
        part = np.asarray(r_["out"]).astype(np.float64)
        # out[p, u*D + col] = partial[u*128 + p, col]
        acc += part.reshape(P, NT, D).transpose(1, 0, 2).reshape(S, D)
    # bv/bo fold: softmax rows sum to 1 => attn @ (V+bv) @ Wo + bo adds bv@Wo + bo
    acc += bv.astype(np.float64) @ Wo.astype(np.float64) + bo.astype(np.float64)
    return acc.astype(np.float32).reshape(1, S, D)


# revision 13
# speedup vs baseline: 1.2320x; 1.0681x over previous
"""TRN2 Bass kernel for nn_MultiHeadAttention_26156350832790.

Multi-head attention: B=1, S=2048, D=2048, H=16 heads (dk=128), causal mask,
fp32 I/O.  Sharded tensor-parallel over 8 NeuronCores: 2 heads per core.

Per-core dataflow (PSUM accumulation always fp32):
  phase 1: full x^T resident in SBUF as bf16 (16 DMAs of [128, 2048] with
           4 KB/partition descriptors); Q^T/K^T [dk, S] evacuated to fp32r,
           V [S, dk] to bf16.  QKV matmuls in bf16.
  phase 2: flash-style attention per (head, 512-wide q-chunk), scores kept
           TRANSPOSED [k, q] (fp32r matmul) so softmax sums come from a
           ones-matmul and the PV matmul needs no P transpose; exp on the
           scalar engine to bf16 probs; causal handled by q-range slicing
           plus one [128,128] triangle mask; normalization via ones-row
           broadcast matmul + reciprocal_approx_fast + vector multiply.
  phase 3: O-projection in bf16 (contraction over the core's 256 head-dims)
           interleaved per chunk, lagging attention by one chunk; partial
           [S, D] written out in bf16 and summed across cores on the host.

Host side: x pre-transposed+tiled, weights pre-tiled, everything cast to
bf16; bq/bk applied in-kernel at Q/K evacuation, bv/bo folded into a
host-side row-vector add (softmax rows sum to 1, so P @ (V + bv) ==
P @ V + bv exactly).
"""

import math
import os
import sys

if "/opt/trn_rl_repo" not in sys.path:
    sys.path.insert(0, "/opt/trn_rl_repo")

import numpy as np
import ml_dtypes

import concourse.bacc as bacc
import concourse.tile as tile
from concourse import mybir
from concourse.bass_utils import run_bass_kernel_spmd

P = 128          # partitions
S = 2048         # sequence
D = 2048         # model dim
NT = 16          # 128-row tiles in S or D
HPC = 2          # heads per core
DK = 128         # head dim
C = 4            # 512-wide chunks
CW = 512         # chunk width
N_CORES = 8
SCALE = 1.0 / math.sqrt(DK)
NEG = -1.0e30

F = mybir.dt.float32
R = mybir.dt.float32r
BF = mybir.dt.bfloat16

_NC = None
last_exec_time_ns = None
_last_in_maps = None


def build():
    nc = bacc.Bacc(None)

    xt = nc.dram_tensor("xt", [P, NT * S], BF, kind="ExternalInput")
    wq = nc.dram_tensor("wq", [P, NT * 2 * DK], BF, kind="ExternalInput")
    wk = nc.dram_tensor("wk", [P, NT * 2 * DK], BF, kind="ExternalInput")
    wv = nc.dram_tensor("wv", [P, NT * 2 * DK], BF, kind="ExternalInput")
    wo = nc.dram_tensor("wo", [P, HPC * D], BF, kind="ExternalInput")
    bqk = nc.dram_tensor("bqk", [P, 4], F, kind="ExternalInput")
    masks = nc.dram_tensor("masks", [P, P], F, kind="ExternalInput")
    onesc = nc.dram_tensor("onesc", [P, 1], BF, kind="ExternalInput")
    onesr = nc.dram_tensor("onesr", [1, P], R, kind="ExternalInput")
    out = nc.dram_tensor("out", [P, NT * D], BF, kind="ExternalOutput")

    Exp = mybir.ActivationFunctionType.Exp

    with tile.TileContext(nc) as tc:
        with (
            tc.tile_pool(name="consts", bufs=1) as consts,
            tc.tile_pool(name="persist", bufs=1) as persist,
        ):
            bqk_sb = consts.tile([P, 4], F)
            mask_sb = consts.tile([P, P], F)
            ones_col = consts.tile([P, 1], BF)
            ones_row = consts.tile([1, P], R)
            # const DMAs are issued inside phase 1's ordered DMA sequence

            # persistent activations
            x_sb = persist.tile([P, NT, S], BF)        # full x^T, t-tiled
            qt_sb = persist.tile([P, HPC * S], BF)     # Q^T per head [dk, S]
            kt_sb = persist.tile([P, HPC * S], BF)     # K^T per head [dk, S]
            v_sb = persist.tile([P, NT * 2 * DK], BF)  # V s-tiles [k, 2*dk]
            attnT_sb = persist.tile([P, HPC * S], BF)  # attn^T per head [dk, S]
            wo_sb = persist.tile([P, HPC * D], BF)     # loaded early, used late

            # ---- phase 1: QKV projections off SBUF-resident x^T
            with (
                tc.tile_pool(name="wqkv", bufs=1) as wqkv,
                tc.tile_pool(name="p1ps", bufs=1, space="PSUM") as p1ps,
            ):
                wq_sb = wqkv.tile([P, NT * 2 * DK], BF)
                wk_sb = wqkv.tile([P, NT * 2 * DK], BF)
                wv_sb = wqkv.tile([P, NT * 2 * DK], BF)
                # DMA order tuned so the first matmuls can start early and
                # the x stream stays ahead of the h-fused t-loop.
                HW = NT * DK  # half the weight columns
                nc.sync.dma_start(out=x_sb[:, 0, :], in_=xt[:, 0:S])
                nc.sync.dma_start(out=wq_sb[:, :HW], in_=wq[:, :HW])
                nc.sync.dma_start(out=wk_sb[:, :HW], in_=wk[:, :HW])
                nc.sync.dma_start(out=wv_sb[:, :HW], in_=wv[:, :HW])
                nc.sync.dma_start(out=x_sb[:, 1, :], in_=xt[:, S : 2 * S])
                nc.sync.dma_start(out=x_sb[:, 2, :], in_=xt[:, 2 * S : 3 * S])
                nc.sync.dma_start(out=wq_sb[:, HW:], in_=wq[:, HW:])
                nc.sync.dma_start(out=wk_sb[:, HW:], in_=wk[:, HW:])
                nc.sync.dma_start(out=wv_sb[:, HW:], in_=wv[:, HW:])
                nc.sync.dma_start(out=bqk_sb, in_=bqk[:])
                for t in range(3, NT):
                    nc.sync.dma_start(
                        out=x_sb[:, t, :], in_=xt[:, t * S : (t + 1) * S]
                    )
                nc.sync.dma_start(out=wo_sb, in_=wo[:])
                nc.sync.dma_start(out=mask_sb, in_=masks[:])
                nc.sync.dma_start(out=ones_col, in_=onesc[:])
                nc.sync.dma_start(out=ones_row, in_=onesr[:])

                for b in range(C):
                    # h-fused t-loop: both heads consume x[t] as it lands,
                    # 8 matmuls per tile so compute outpaces the DMA stream
                    ps = {}
                    for h in range(HPC):
                        ps[h] = (
                            p1ps.tile([P, CW], F, name=f"qps{h}", tag=f"qps{h}"),
                            p1ps.tile([P, CW], F, name=f"kps{h}", tag=f"kps{h}"),
                            p1ps.tile([P, 2 * DK], F, name=f"vps{h}0", tag=f"vps{h}0"),
                            p1ps.tile([P, 2 * DK], F, name=f"vps{h}1", tag=f"vps{h}1"),
                        )
                    for t in range(NT):
                        st = t == 0
                        sp = t == NT - 1
                        cs = slice(b * CW, (b + 1) * CW)
                        for h in range(HPC):
                            qps, kps, vps0, vps1 = ps[h]
                            u = 4 * b + 2 * h
                            nc.tensor.matmul(
                                qps[:],
                                wq_sb[:, t * 2 * DK + h * DK : t * 2 * DK + (h + 1) * DK],
                                x_sb[:, t, cs],
                                start=st,
                                stop=sp,
                            )
                            nc.tensor.matmul(
                                vps0[:],
                                x_sb[:, t, u * DK : (u + 1) * DK],
                                wv_sb[:, t * 2 * DK : (t + 1) * 2 * DK],
                                start=st,
                                stop=sp,
                            )
                            nc.tensor.matmul(
                                kps[:],
                                wk_sb[:, t * 2 * DK + h * DK : t * 2 * DK + (h + 1) * DK],
                                x_sb[:, t, cs],
                                start=st,
                                stop=sp,
                            )
                            nc.tensor.matmul(
                                vps1[:],
                                x_sb[:, t, (u + 1) * DK : (u + 2) * DK],
                                wv_sb[:, t * 2 * DK : (t + 1) * 2 * DK],
                                start=st,
                                stop=sp,
                            )
                    for h in range(HPC):
                        qps, kps, vps0, vps1 = ps[h]
                        u = 4 * b + 2 * h
                        with nc.allow_low_precision(reason="fp32r evac"):
                            nc.scalar.add(
                                qt_sb[:, h * S + b * CW : h * S + (b + 1) * CW],
                                qps[:],
                                bqk_sb[:, h : h + 1],
                            )
                            nc.scalar.add(
                                kt_sb[:, h * S + b * CW : h * S + (b + 1) * CW],
                                kps[:],
                                bqk_sb[:, 2 + h : 3 + h],
                            )
                        with nc.allow_low_precision(reason="bf16 V evac"):
                            nc.vector.tensor_copy(
                                v_sb[:, u * 2 * DK : (u + 1) * 2 * DK], vps0[:]
                            )
                            nc.vector.tensor_copy(
                                v_sb[:, (u + 1) * 2 * DK : (u + 2) * 2 * DK],
                                vps1[:],
                            )

            # ---- phases 2+3: causal attention (scores transposed [k, q])
            # with the previous chunk's O-projection units interleaved at
            # j-tile granularity so the in-order tensor queue always has
            # ready work while the scalar engine paces the exp chain.
            with (
                tc.tile_pool(name="ps23", bufs=1, space="PSUM") as ps23,
                tc.tile_pool(name="ptp", bufs=4) as ptp,
                tc.tile_pool(name="ssp", bufs=2) as ssp,
                tc.tile_pool(name="bcp", bufs=2) as bcp,
                tc.tile_pool(name="outp", bufs=3) as outp,
            ):
                ot_cur = [None]

                def emit_ounit(u, e, final=False):
                    if e == 0:
                        ot_cur[0] = outp.tile([P, D], BF, name="ot", tag="ot")
                    ot = ot_cur[0]
                    o3 = ps23.tile([P, CW], F, name="o3", tag="o3", bufs=2)
                    for h in range(HPC):
                        nc.tensor.matmul(
                            o3[:],
                            attnT_sb[:, h * S + u * P : h * S + (u + 1) * P],
                            wo_sb[:, h * D + e * CW : h * D + (e + 1) * CW],
                            start=(h == 0),
                            stop=(h == HPC - 1),
                        )
                    with nc.allow_low_precision(reason="bf16 out"):
                        if final:
                            nc.scalar.copy(ot[:, e * CW : (e + 1) * CW], o3[:])
                        else:
                            nc.vector.tensor_copy(
                                ot[:, e * CW : (e + 1) * CW], o3[:]
                            )
                    if e == C - 1:
                        nc.sync.dma_start(
                            out=out[:, u * D : (u + 1) * D], in_=ot[:]
                        )

                for c in range(C):
                    # O-units of the previous chunk, spread over this chunk
                    units = (
                        [(u, e) for u in range(4 * (c - 1), 4 * c) for e in range(C)]
                        if c > 0
                        else []
                    )
                    nslots = HPC * (4 * c + 4)
                    slot = 0
                    emitted = 0

                    def pace():
                        nonlocal slot, emitted
                        slot += 1
                        want = (len(units) * slot) // nslots
                        while emitted < want:
                            emit_ounit(*units[emitted])
                            emitted += 1

                    for h in range(HPC):
                        jmax = 4 * c + 3
                        sum_ps = ps23.tile(
                            [1, CW], F, name="sum_ps", tag="B", bufs=1
                        )
                        o_ps = ps23.tile(
                            [P, CW], F, name="o_ps", tag="Cc", bufs=2
                        )
                        lag = None  # pending PV/sums
                        for j in range(jmax + 1):
                            t = j - 4 * c
                            lo = P * t if t >= 0 else 0
                            sc = ps23.tile(
                                [P, CW], F, name="sc", tag="A", bufs=3
                            )
                            nc.tensor.matmul(
                                sc[:, lo:],
                                kt_sb[:, h * S + j * P : h * S + (j + 1) * P],
                                qt_sb[:, h * S + c * CW + lo : h * S + (c + 1) * CW],
                                start=True,
                                stop=True,
                            )
                            if t >= 0:
                                nc.vector.tensor_add(
                                    sc[:, lo : lo + P],
                                    sc[:, lo : lo + P],
                                    mask_sb[:],
                                )
                            pt = ptp.tile([P, CW], BF, name="pt")
                            nc.scalar.activation(
                                pt[:, lo:], sc[:, lo:], Exp, scale=SCALE
                            )
                            if lag is not None:
                                lpt, llo, lst, lsp, lj = lag
                                nc.tensor.matmul(
                                    o_ps[:, llo:],
                                    v_sb[:, lj * 2 * DK + h * DK : lj * 2 * DK + (h + 1) * DK],
                                    lpt[:, llo:],
                                    start=lst,
                                    stop=lsp,
                                )
                                nc.tensor.matmul(
                                    sum_ps[:, llo:],
                                    ones_col[:],
                                    lpt[:, llo:],
                                    start=lst,
                                    stop=lsp,
                                )
                            lag = (pt, lo, j == 0, j == jmax, j)
                            pace()
                        lpt, llo, lst, lsp, lj = lag
                        nc.tensor.matmul(
                            o_ps[:, llo:],
                            v_sb[:, lj * 2 * DK + h * DK : lj * 2 * DK + (h + 1) * DK],
                            lpt[:, llo:],
                            start=lst,
                            stop=lsp,
                        )
                        nc.tensor.matmul(
                            sum_ps[:, llo:],
                            ones_col[:],
                            lpt[:, llo:],
                            start=lst,
                            stop=lsp,
                        )
                        # normalization: 1/rowsum broadcast to [P, CW],
                        # all off the tensor queue (gpsimd does the
                        # partition broadcast, vector the wide reciprocal)
                        ss = ssp.tile([1, CW], F, name="ss")
                        nc.vector.tensor_copy(ss[:], sum_ps[:])
                        bsum = bcp.tile([P, CW], F, name="bsum", tag="bsum")
                        nc.gpsimd.partition_broadcast(bsum[:], ss[:])
                        bc = bcp.tile([P, CW], F, name="bc", tag="bc")
                        nc.vector.reciprocal_approx_fast(out=bc[:], in_=bsum[:])
                        with nc.allow_low_precision(reason="bf16 attnT"):
                            nc.vector.tensor_mul(
                                attnT_sb[:, h * S + c * CW : h * S + (c + 1) * CW],
                                o_ps[:],
                                bc[:],
                            )
                    while emitted < len(units):
                        emit_ounit(*units[emitted])
                        emitted += 1
                # final chunk's O-projection (evacs on scalar — exp is done)
                for u in range(4 * (C - 1), 4 * C):
                    for e in range(C):
                        emit_ounit(u, e, final=True)

    nc.compile()
    return nc


def _tile_weight_cols(w_slice: np.ndarray) -> np.ndarray:
    """[2048, 256] -> [128, 16*256] with block t = rows [128t, 128t+128)."""
    return np.ascontiguousarray(
        w_slice.reshape(NT, P, 2 * DK).transpose(1, 0, 2).reshape(P, NT * 2 * DK)
    )


def _make_masks() -> np.ndarray:
    """[128,128] additive causal triangle: 0 where p <= f, -1e30 where p > f."""
    p = np.arange(P)[:, None]
    f = np.arange(P)[None, :]
    return np.where(p <= f, 0.0, NEG).astype(np.float32)


def kernel(x, Wq, bq, Wk, bk, Wv, bv, Wo, bo):
    global _NC, last_exec_time_ns, _last_in_maps

    BFH = ml_dtypes.bfloat16
    x = np.asarray(x, dtype=np.float32)
    Wq = np.asarray(Wq, dtype=np.float32)
    Wk = np.asarray(Wk, dtype=np.float32)
    Wv = np.asarray(Wv, dtype=np.float32)
    Wo = np.asarray(Wo, dtype=np.float32)
    bq = np.asarray(bq, dtype=np.float32)
    bk = np.asarray(bk, dtype=np.float32)
    bv = np.asarray(bv, dtype=np.float32)
    bo = np.asarray(bo, dtype=np.float32)

    if _NC is None:
        _NC = build()

    # x^T tiled: xt[p, t*S + s] = x[s, t*128 + p]
    xt = np.ascontiguousarray(
        x[0].T.reshape(NT, P, S).transpose(1, 0, 2).reshape(P, NT * S)
    ).astype(BFH)
    masks = _make_masks()

    in_maps = []
    for i in range(N_CORES):
        cs = slice(2 * DK * i, 2 * DK * (i + 1))
        bqk_i = np.stack(
            [
                bq[2 * DK * i : 2 * DK * i + DK],
                bq[2 * DK * i + DK : 2 * DK * (i + 1)],
                bk[2 * DK * i : 2 * DK * i + DK],
                bk[2 * DK * i + DK : 2 * DK * (i + 1)],
            ],
            axis=1,
        ).astype(np.float32)
        wo_i = np.ascontiguousarray(
            Wo[cs, :].reshape(HPC, P, D).transpose(1, 0, 2).reshape(P, HPC * D)
        ).astype(BFH)
        in_maps.append(
            {
                "xt": xt,
                "wq": _tile_weight_cols(Wq[:, cs]).astype(BFH),
                "wk": _tile_weight_cols(Wk[:, cs]).astype(BFH),
                "wv": _tile_weight_cols(Wv[:, cs]).astype(BFH),
                "wo": wo_i,
                "bqk": bqk_i,
                "masks": masks,
                "onesc": np.ones((P, 1), BFH),
                "onesr": np.ones((1, P), np.float32),
            }
        )

    _last_in_maps = in_maps
    trace = bool(int(os.environ.get("BASS_TRACE", "0") or "0"))
    if trace:
        try:
            import ntff_shim

            ntff_shim.install()
        except Exception:
            pass

    res = run_bass_kernel_spmd(
        _NC, in_maps, core_ids=list(range(N_CORES)), trace=trace
    )
    last_exec_time_ns = res.exec_time_ns

    acc = np.zeros((S, D), dtype=np.float64)
    for r_ in res.results:
        part = np.asarray(r_["out"]).astype(np.float64)
        # out[p, u*D + col] = partial[u*128 + p, col]
        acc += part.reshape(P, NT, D).transpose(1, 0, 2).reshape(S, D)
    # bv/bo fold: softmax rows sum to 1 => attn @ (V+bv) @ Wo + bo adds bv@Wo + bo
    acc += bv.astype(np.float64) @ Wo.astype(np.float64) + bo.astype(np.float64)
    return acc.astype(np.float32).reshape(1, S, D)


# revision 23
# speedup vs baseline: 1.2591x; 1.0220x over previous
"""TRN2 Bass kernel for nn_MultiHeadAttention_26156350832790.

Multi-head attention: B=1, S=2048, D=2048, H=16 heads (dk=128), causal mask,
fp32 I/O.  Sharded tensor-parallel over 8 NeuronCores: 2 heads per core.

Per-core dataflow (PSUM accumulation always fp32):
  phase 1: full x^T resident in SBUF as bf16 (16 DMAs of [128, 2048] with
           4 KB/partition descriptors); Q^T/K^T [dk, S] evacuated to fp32r,
           V [S, dk] to bf16.  QKV matmuls in bf16.
  phase 2: flash-style attention per (head, 512-wide q-chunk), scores kept
           TRANSPOSED [k, q] (fp32r matmul) so softmax sums come from a
           ones-matmul and the PV matmul needs no P transpose; exp on the
           scalar engine to bf16 probs; causal handled by q-range slicing
           plus one [128,128] triangle mask; normalization via ones-row
           broadcast matmul + reciprocal_approx_fast + vector multiply.
  phase 3: O-projection in bf16 (contraction over the core's 256 head-dims)
           interleaved per chunk, lagging attention by one chunk; partial
           [S, D] written out in bf16 and summed across cores on the host.

Host side: x pre-transposed+tiled, weights pre-tiled, everything cast to
bf16; bq/bk applied in-kernel at Q/K evacuation, bv/bo folded into a
host-side row-vector add (softmax rows sum to 1, so P @ (V + bv) ==
P @ V + bv exactly).
"""

import math
import os
import sys

if "/opt/trn_rl_repo" not in sys.path:
    sys.path.insert(0, "/opt/trn_rl_repo")

import numpy as np
import ml_dtypes

import concourse.bacc as bacc
import concourse.tile as tile
from concourse import mybir
from concourse.bass_utils import run_bass_kernel_spmd

P = 128          # partitions
S = 2048         # sequence
D = 2048         # model dim
NT = 16          # 128-row tiles in S or D
HPC = 2          # heads per core
DK = 128         # head dim
C = 4            # 512-wide chunks
CW = 512         # chunk width
N_CORES = 8
SCALE = 1.0 / math.sqrt(DK)
NEG = -1.0e30

F = mybir.dt.float32
R = mybir.dt.float32r
BF = mybir.dt.bfloat16

_NC = None
last_exec_time_ns = None
_last_in_maps = None


def build():
    nc = bacc.Bacc(None)

    xt = nc.dram_tensor("xt", [P, NT * S], BF, kind="ExternalInput")
    wq = nc.dram_tensor("wq", [P, NT * 2 * DK], BF, kind="ExternalInput")
    wk = nc.dram_tensor("wk", [P, NT * 2 * DK], BF, kind="ExternalInput")
    wv = nc.dram_tensor("wv", [P, NT * 2 * DK], BF, kind="ExternalInput")
    wo = nc.dram_tensor("wo", [P, HPC * D], BF, kind="ExternalInput")
    bqk = nc.dram_tensor("bqk", [P, 4], F, kind="ExternalInput")
    masks = nc.dram_tensor("masks", [P, P], F, kind="ExternalInput")
    onesc = nc.dram_tensor("onesc", [P, 1], BF, kind="ExternalInput")
    onesr = nc.dram_tensor("onesr", [1, P], R, kind="ExternalInput")
    out = nc.dram_tensor("out", [P, NT * D], BF, kind="ExternalOutput")

    Exp = mybir.ActivationFunctionType.Exp

    with tile.TileContext(nc) as tc:
        with (
            tc.tile_pool(name="consts", bufs=1) as consts,
            tc.tile_pool(name="persist", bufs=1) as persist,
        ):
            bqk_sb = consts.tile([P, 4], F)
            mask_sb = consts.tile([P, P], F)
            ones_col = consts.tile([P, 1], BF)
            pbwarm = consts.tile([P, 1], F)
            # const DMAs are issued inside phase 1's ordered DMA sequence

            # persistent activations
            x_sb = persist.tile([P, NT, S], BF)        # full x^T, t-tiled
            qt_sb = persist.tile([P, HPC * S], BF)     # Q^T per head [dk, S]
            kt_sb = persist.tile([P, HPC * S], BF)     # K^T per head [dk, S]
            v_sb = persist.tile([P, NT * 2 * DK], BF)  # V s-tiles [k, 2*dk]
            attnT_sb = persist.tile([P, HPC * S], BF)  # attn^T per head [dk, S]
            wo_sb = persist.tile([P, HPC * D], BF)     # loaded early, used late

            # ---- phase 1: QKV projections off SBUF-resident x^T
            with (
                tc.tile_pool(name="wqkv", bufs=1) as wqkv,
                tc.tile_pool(name="p1ps", bufs=1, space="PSUM") as p1ps,
            ):
                wq_sb = wqkv.tile([P, NT * 2 * DK], BF)
                wk_sb = wqkv.tile([P, NT * 2 * DK], BF)
                wv_sb = wqkv.tile([P, NT * 2 * DK], BF)
                # DMA order tuned so the first matmuls can start early and
                # the x stream stays ahead of the h-fused t-loop.
                HW = NT * DK  # half the weight columns
                nc.sync.dma_start(out=x_sb[:, 0, :], in_=xt[:, 0:S])
                nc.sync.dma_start(out=wq_sb[:, :HW], in_=wq[:, :HW])
                nc.sync.dma_start(out=wk_sb[:, :HW], in_=wk[:, :HW])
                nc.sync.dma_start(out=wv_sb[:, :HW], in_=wv[:, :HW])
                for t in range(1, 6):
                    nc.sync.dma_start(
                        out=x_sb[:, t, :], in_=xt[:, t * S : (t + 1) * S]
                    )
                nc.sync.dma_start(out=bqk_sb, in_=bqk[:])
                nc.sync.dma_start(out=wq_sb[:, HW:], in_=wq[:, HW:])
                nc.sync.dma_start(out=wk_sb[:, HW:], in_=wk[:, HW:])
                nc.sync.dma_start(out=wv_sb[:, HW:], in_=wv[:, HW:])
                for t in range(6, NT):
                    nc.sync.dma_start(
                        out=x_sb[:, t, :], in_=xt[:, t * S : (t + 1) * S]
                    )
                nc.sync.dma_start(out=wo_sb, in_=wo[:])
                nc.sync.dma_start(out=mask_sb, in_=masks[:])
                nc.sync.dma_start(out=ones_col, in_=onesc[:])
                # warm the gpsimd custom-op library while it is idle so the
                # first real partition_broadcast doesn't pay the load
                nc.gpsimd.partition_broadcast(pbwarm[:], bqk_sb[0:1, 0:1])

                for b in range(C):
                    # h-fused t-loop: both heads consume x[t] as it lands,
                    # 8 matmuls per tile so compute outpaces the DMA stream
                    ps = {}
                    for h in range(HPC):
                        ps[h] = (
                            p1ps.tile([P, CW], F, name=f"qps{h}", tag=f"qps{h}"),
                            p1ps.tile([P, CW], F, name=f"kps{h}", tag=f"kps{h}"),
                            p1ps.tile([P, 2 * DK], F, name=f"vps{h}0", tag=f"vps{h}0"),
                            p1ps.tile([P, 2 * DK], F, name=f"vps{h}1", tag=f"vps{h}1"),
                        )
                    for t in range(NT):
                        st = t == 0
                        sp = t == NT - 1
                        cs = slice(b * CW, (b + 1) * CW)
                        for h in range(HPC):
                            qps, kps, vps0, vps1 = ps[h]
                            u = 4 * b + 2 * h
                            nc.tensor.matmul(
                                qps[:],
                                wq_sb[:, t * 2 * DK + h * DK : t * 2 * DK + (h + 1) * DK],
                                x_sb[:, t, cs],
                                start=st,
                                stop=sp,
                            )
                            nc.tensor.matmul(
                                vps0[:],
                                x_sb[:, t, u * DK : (u + 1) * DK],
                                wv_sb[:, t * 2 * DK : (t + 1) * 2 * DK],
                                start=st,
                                stop=sp,
                            )
                            nc.tensor.matmul(
                                kps[:],
                                wk_sb[:, t * 2 * DK + h * DK : t * 2 * DK + (h + 1) * DK],
                                x_sb[:, t, cs],
                                start=st,
                                stop=sp,
                            )
                            nc.tensor.matmul(
                                vps1[:],
                                x_sb[:, t, (u + 1) * DK : (u + 2) * DK],
                                wv_sb[:, t * 2 * DK : (t + 1) * 2 * DK],
                                start=st,
                                stop=sp,
                            )
                    for h in range(HPC):
                        qps, kps, vps0, vps1 = ps[h]
                        u = 4 * b + 2 * h
                        with nc.allow_low_precision(reason="bf16 evac"):
                            nc.vector.tensor_scalar_add(
                                qt_sb[:, h * S + b * CW : h * S + (b + 1) * CW],
                                qps[:],
                                bqk_sb[:, h : h + 1],
                            )
                            nc.vector.tensor_scalar_add(
                                kt_sb[:, h * S + b * CW : h * S + (b + 1) * CW],
                                kps[:],
                                bqk_sb[:, 2 + h : 3 + h],
                            )
                            nc.vector.tensor_copy(
                                v_sb[:, u * 2 * DK : (u + 1) * 2 * DK], vps0[:]
                            )
                            nc.vector.tensor_copy(
                                v_sb[:, (u + 1) * 2 * DK : (u + 2) * 2 * DK],
                                vps1[:],
                            )

            # ---- phases 2+3: causal attention (scores transposed [k, q])
            # with the previous chunk's O-projection units interleaved at
            # j-tile granularity so the in-order tensor queue always has
            # ready work while the scalar engine paces the exp chain.
            with (
                tc.tile_pool(name="ps23", bufs=1, space="PSUM") as ps23,
                tc.tile_pool(name="ptp", bufs=4) as ptp,
                tc.tile_pool(name="ssp", bufs=2) as ssp,
                tc.tile_pool(name="bcp", bufs=2) as bcp,
                tc.tile_pool(name="outp", bufs=3) as outp,
            ):
                ot_cur = [None]

                def emit_ounit(u, e, final=False):
                    if e == 0:
                        ot_cur[0] = outp.tile([P, D], BF, name="ot", tag="ot")
                    ot = ot_cur[0]
                    o3 = ps23.tile([P, CW], F, name="o3", tag="o3", bufs=2)
                    for h in range(HPC):
                        nc.tensor.matmul(
                            o3[:],
                            attnT_sb[:, h * S + u * P : h * S + (u + 1) * P],
                            wo_sb[:, h * D + e * CW : h * D + (e + 1) * CW],
                            start=(h == 0),
                            stop=(h == HPC - 1),
                        )
                    with nc.allow_low_precision(reason="bf16 out"):
                        if final and e % 2 == 0:
                            nc.scalar.copy(ot[:, e * CW : (e + 1) * CW], o3[:])
                        else:
                            nc.vector.tensor_copy(
                                ot[:, e * CW : (e + 1) * CW], o3[:]
                            )
                    if e == C - 1:
                        nc.sync.dma_start(
                            out=out[:, u * D : (u + 1) * D], in_=ot[:]
                        )

                for c in range(C):
                    # O-units of the previous chunk, spread over this chunk.
                    # The first 3 slots emit nothing so the previous chunk's
                    # normalization chain has time to produce attnT.
                    units = (
                        [(u, e) for u in range(4 * (c - 1), 4 * c) for e in range(C)]
                        if c > 0
                        else []
                    )
                    nslots = HPC * (4 * c + 4)
                    slot = 0
                    emitted = 0

                    def pace():
                        nonlocal slot, emitted
                        slot += 1
                        want = (len(units) * max(0, slot - 3)) // max(1, nslots - 3)
                        while emitted < want:
                            emit_ounit(*units[emitted])
                            emitted += 1

                    for h in range(HPC):
                        jmax = 4 * c + 3
                        sum_ps = ps23.tile(
                            [1, CW], F, name="sum_ps", tag="B", bufs=1
                        )
                        o_ps = ps23.tile(
                            [P, CW], F, name="o_ps", tag="Cc", bufs=2
                        )
                        lag = None  # pending PV/sums
                        for j in range(jmax + 1):
                            t = j - 4 * c
                            lo = P * t if t >= 0 else 0
                            sc = ps23.tile(
                                [P, CW], F, name="sc", tag="A", bufs=3
                            )
                            nc.tensor.matmul(
                                sc[:, lo:],
                                kt_sb[:, h * S + j * P : h * S + (j + 1) * P],
                                qt_sb[:, h * S + c * CW + lo : h * S + (c + 1) * CW],
                                start=True,
                                stop=True,
                            )
                            if t >= 0:
                                nc.vector.tensor_add(
                                    sc[:, lo : lo + P],
                                    sc[:, lo : lo + P],
                                    mask_sb[:],
                                )
                            pt = ptp.tile([P, CW], BF, name="pt")
                            nc.scalar.activation(
                                pt[:, lo:], sc[:, lo:], Exp, scale=SCALE
                            )
                            if lag is not None:
                                lpt, llo, lst, lsp, lj = lag
                                nc.tensor.matmul(
                                    o_ps[:, llo:],
                                    v_sb[:, lj * 2 * DK + h * DK : lj * 2 * DK + (h + 1) * DK],
                                    lpt[:, llo:],
                                    start=lst,
                                    stop=lsp,
                                )
                                nc.tensor.matmul(
                                    sum_ps[:, llo:],
                                    ones_col[:],
                                    lpt[:, llo:],
                                    start=lst,
                                    stop=lsp,
                                )
                            lag = (pt, lo, j == 0, j == jmax, j)
                            pace()
                        lpt, llo, lst, lsp, lj = lag
                        nc.tensor.matmul(
                            o_ps[:, llo:],
                            v_sb[:, lj * 2 * DK + h * DK : lj * 2 * DK + (h + 1) * DK],
                            lpt[:, llo:],
                            start=lst,
                            stop=lsp,
                        )
                        nc.tensor.matmul(
                            sum_ps[:, llo:],
                            ones_col[:],
                            lpt[:, llo:],
                            start=lst,
                            stop=lsp,
                        )
                        # normalization: 1/rowsum broadcast to [P, CW],
                        # all off the tensor queue (gpsimd does the
                        # partition broadcast, vector the wide reciprocal)
                        ss = ssp.tile([1, CW], F, name="ss")
                        nc.vector.tensor_copy(ss[:], sum_ps[:])
                        bsum = bcp.tile([P, CW], F, name="bsum", tag="bsum")
                        nc.gpsimd.partition_broadcast(bsum[:], ss[:])
                        bc = bcp.tile([P, CW], F, name="bc", tag="bc")
                        nc.vector.reciprocal_approx_fast(out=bc[:], in_=bsum[:])
                        with nc.allow_low_precision(reason="bf16 attnT"):
                            nc.vector.tensor_mul(
                                attnT_sb[:, h * S + c * CW : h * S + (c + 1) * CW],
                                o_ps[:],
                                bc[:],
                            )
                    while emitted < len(units):
                        emit_ounit(*units[emitted])
                        emitted += 1
                # final chunk's O-projection; casts alternate scalar/vector
                for u in range(4 * (C - 1), 4 * C):
                    for e in range(C):
                        emit_ounit(u, e, final=True)

    nc.compile()
    return nc


def _tile_weight_cols(w_slice: np.ndarray) -> np.ndarray:
    """[2048, 256] -> [128, 16*256] with block t = rows [128t, 128t+128)."""
    return np.ascontiguousarray(
        w_slice.reshape(NT, P, 2 * DK).transpose(1, 0, 2).reshape(P, NT * 2 * DK)
    )


def _make_masks() -> np.ndarray:
    """[128,128] additive causal triangle: 0 where p <= f, -1e30 where p > f."""
    p = np.arange(P)[:, None]
    f = np.arange(P)[None, :]
    return np.where(p <= f, 0.0, NEG).astype(np.float32)


def kernel(x, Wq, bq, Wk, bk, Wv, bv, Wo, bo):
    global _NC, last_exec_time_ns, _last_in_maps

    BFH = ml_dtypes.bfloat16
    x = np.asarray(x, dtype=np.float32)
    Wq = np.asarray(Wq, dtype=np.float32)
    Wk = np.asarray(Wk, dtype=np.float32)
    Wv = np.asarray(Wv, dtype=np.float32)
    Wo = np.asarray(Wo, dtype=np.float32)
    bq = np.asarray(bq, dtype=np.float32)
    bk = np.asarray(bk, dtype=np.float32)
    bv = np.asarray(bv, dtype=np.float32)
    bo = np.asarray(bo, dtype=np.float32)

    if _NC is None:
        _NC = build()

    # x^T tiled: xt[p, t*S + s] = x[s, t*128 + p]
    xt = np.ascontiguousarray(
        x[0].T.reshape(NT, P, S).transpose(1, 0, 2).reshape(P, NT * S)
    ).astype(BFH)
    masks = _make_masks()

    in_maps = []
    for i in range(N_CORES):
        cs = slice(2 * DK * i, 2 * DK * (i + 1))
        bqk_i = np.stack(
            [
                bq[2 * DK * i : 2 * DK * i + DK],
                bq[2 * DK * i + DK : 2 * DK * (i + 1)],
                bk[2 * DK * i : 2 * DK * i + DK],
                bk[2 * DK * i + DK : 2 * DK * (i + 1)],
            ],
            axis=1,
        ).astype(np.float32)
        wo_i = np.ascontiguousarray(
            Wo[cs, :].reshape(HPC, P, D).transpose(1, 0, 2).reshape(P, HPC * D)
        ).astype(BFH)
        in_maps.append(
            {
                "xt": xt,
                "wq": _tile_weight_cols(Wq[:, cs]).astype(BFH),
                "wk": _tile_weight_cols(Wk[:, cs]).astype(BFH),
                "wv": _tile_weight_cols(Wv[:, cs]).astype(BFH),
                "wo": wo_i,
                "bqk": bqk_i,
                "masks": masks,
                "onesc": np.ones((P, 1), BFH),
                "onesr": np.ones((1, P), np.float32),
            }
        )

    _last_in_maps = in_maps
    trace = bool(int(os.environ.get("BASS_TRACE", "0") or "0"))
    if trace:
        try:
            import ntff_shim

            ntff_shim.install()
        except Exception:
            pass

    res = run_bass_kernel_spmd(
        _NC, in_maps, core_ids=list(range(N_CORES)), trace=trace
    )
    last_exec_time_ns = res.exec_time_ns

    acc = np.zeros((S, D), dtype=np.float64)
    for r_ in res.results:
        part = np.asarray(r_["out"]).astype(np.float64)
        # out[p, u*D + col] = partial[u*128 + p, col]
        acc += part.reshape(P, NT, D).transpose(1, 0, 2).reshape(S, D)
    # bv/bo fold: softmax rows sum to 1 => attn @ (V+bv) @ Wo + bo adds bv@Wo + bo
    acc += bv.astype(np.float64) @ Wo.astype(np.float64) + bo.astype(np.float64)
    return acc.astype(np.float32).reshape(1, S, D)
